# revision 7
# baseline (speedup 1.0000x reference)
"""Trainium2 Bass kernel for nn_CrossAttention (efficient-attention form).

Reference computation per batch b:
    K = softmax(x2, axis=-1)           # over D
    Q = softmax(x2, axis=1)            # over N
    out = ((x @ K.T) @ Q) @ W.T + b

Reassociated (matmuls are associative; both softmaxes share exp(x2)):
    E  = exp(x2)                       # one exp serves both softmaxes
    K  = E * (1/rowsum(E))             # per-row scale
    G  = K.T @ Q = (K.T @ E) * diag(1/colsum(E))
    out = x @ (G @ W.T) + b            # single [N,D]@[D,D] matmul on x

This turns 2*N^2*D MACs into ~2*N*D^2 (4x fewer at N=2048, D=512), and the
colsum is recovered for free: rows of K sum to 1, so rowsum(K.T @ E over d)
= colsum(E).  Batch dim B=8 is sharded across the 8 cores (data parallel).
"""

import sys

import numpy as np

if "/opt/trn_rl_repo" not in sys.path:
    sys.path.insert(0, "/opt/trn_rl_repo")

import concourse.bass as bass
import concourse.mybir as mybir
import concourse.tile as tile
from concourse import bacc
from concourse.bass import ts
from concourse.bass_utils import run_bass_kernel_spmd
from concourse.masks import make_identity

B, N, D = 8, 2048, 512
P = 128
NT = N // P  # 16 row tiles
DC = D // P  # 4 column chunks of D
F32 = mybir.dt.float32
F32R = mybir.dt.float32r

_CACHE = {}


def _build_nc():
    nc = bacc.Bacc("TRN2", target_bir_lowering=False, debug=False)
    x_d = nc.declare_dram_parameter("x", [N, D], F32, isOutput=False)
    x2_d = nc.declare_dram_parameter("x2", [N, D], F32, isOutput=False)
    w_d = nc.declare_dram_parameter("W", [D, D], F32, isOutput=False)
    b_d = nc.declare_dram_parameter("b", [D], F32, isOutput=False)
    out_d = nc.declare_dram_parameter("out", [N, D], F32, isOutput=True)

    x_t = x_d[:].rearrange("(i p) d -> i p d", p=P)
    x2_t = x2_d[:].rearrange("(i p) d -> i p d", p=P)
    w_t = w_d[:].rearrange("(j p) d -> p j d", p=P)
    out_t = out_d[:].rearrange("(i p) d -> i p d", p=P)

    with tile.TileContext(nc) as tc:
        with (
            tc.tile_pool(name="big", bufs=1) as big,
            tc.tile_pool(name="stage", bufs=4) as stage,
            tc.tile_pool(name="small", bufs=1) as small,
            tc.tile_pool(name="stats", bufs=4) as stats,
            tc.tile_pool(name="outp", bufs=3) as outp,
            tc.tile_pool(name="psA", bufs=1, space="PSUM") as psA,
            tc.tile_pool(name="psT", bufs=2, space="PSUM") as psT,
            tc.tile_pool(name="psO", bufs=2, space="PSUM") as psO,
        ):
            # ---- persistent SBUF tensors
            e_all = big.tile([P, NT, D], F32R, tag="e_all")  # exp(x2), n-major
            k_all = big.tile([P, NT, D], F32R, tag="k_all")  # K softmax rows
            xt_all = big.tile([P, DC, N], F32R, tag="xt_all")  # x^T, d-major
            mt_all = big.tile([P, DC, D], F32R, tag="mt_all")  # (K^T E)^T chunks
            wt_all = big.tile([P, DC, D], F32, tag="wt_all")  # W^T
            v_all = big.tile([P, DC, D], F32R, tag="v_all")  # diag(s) @ W^T
            c_all = big.tile([P, DC, D], F32R, tag="c_all")  # G @ W^T
            wn_all = big.tile([P, DC, D], F32, tag="wn_all")  # W natural
            ident = small.tile([P, P], F32, tag="ident")
            bias_bc = small.tile([P, D], F32, tag="bias_bc")

            make_identity(nc, ident)
            b_ap = b_d[:]
            nc.gpsimd.dma_start(
                out=bias_bc,
                in_=bass.AP(tensor=b_ap.tensor, offset=b_ap.offset,
                            ap=[[0, P]] + list(b_ap.ap)),
            )
            nc.sync.dma_start(out=wn_all, in_=w_t)

            # psum accumulator for M''^T = (K^T E)^T: 4 chunks x [128, 512]
            ps_m = psA.tile([P, DC, D], F32, tag="ps_m")

            # ---- stream the 16 row-tiles of x2/x
            for i in range(NT):
                x2_s = stage.tile([P, D], F32, tag="x2_s")
                nc.sync.dma_start(out=x2_s, in_=x2_t[i])
                e_i = e_all[:, i, :]
                k_i = k_all[:, i, :]
                rs = stats.tile([P, 1], F32, tag="rs")
                nc.scalar.activation(
                    out=e_i, in_=x2_s,
                    func=mybir.ActivationFunctionType.Exp,
                    accum_out=rs,
                )
                rr = stats.tile([P, 1], F32, tag="rr")
                nc.vector.reciprocal(out=rr, in_=rs)
                nc.vector.tensor_scalar_mul(k_i, e_i.bitcast(F32), rr)
                # M''^T[d',d] += sum_n E[n,d'] K[n,d]
                for j in range(DC):
                    nc.tensor.matmul(
                        ps_m[:, j, :],
                        lhsT=e_i[:, ts(j, P)],
                        rhs=k_i,
                        start=(i == 0), stop=(i == NT - 1),
                    )
                # x^T via PE transpose, interleaved to keep PE warm
                x_s = stage.tile([P, D], F32, tag="x_s")
                nc.sync.dma_start(out=x_s, in_=x_t[i])
                for j in range(DC):
                    pt = psT.tile([P, P], F32, tag="pt")
                    nc.tensor.transpose(pt, x_s[:, ts(j, P)], ident)
                    nc.vector.tensor_copy(xt_all[:, j, ts(i, P)], pt)
                # W^T via PE transpose, spread over early iterations
                if 1 <= i <= 4:
                    jw = i - 1
                    for kw in range(DC):
                        pw = psT.tile([P, P], F32, tag="pt")
                        nc.tensor.transpose(pw, wn_all[:, jw, ts(kw, P)], ident)
                        nc.vector.tensor_copy(wt_all[:, kw, ts(jw, P)], pw)

            # ---- normalize: s = 1/colsum(E); colsum = rowsum of M''^T chunks
            for j in range(DC):
                cs = stats.tile([P, 1], F32, tag="cs")
                nc.vector.tensor_scalar(
                    out=mt_all[:, j, :], in0=ps_m[:, j, :],
                    scalar1=1.0, scalar2=0.0,
                    op0=mybir.AluOpType.mult,
                    op1=mybir.AluOpType.add,
                    accum_out=cs,
                )
                sj = stats.tile([P, 1], F32, tag="sj")
                nc.vector.reciprocal(out=sj, in_=cs)
                nc.vector.tensor_scalar_mul(v_all[:, j, :], wt_all[:, j, :], sj)

            # ---- C = M'' diag(s) W^T  ([D, D])
            for k in range(DC):
                pc = psO.tile([P, D], F32, tag="po")
                for j in range(DC):
                    nc.tensor.matmul(
                        pc,
                        lhsT=mt_all[:, j, ts(k, P)],
                        rhs=v_all[:, j, :],
                        start=(j == 0), stop=(j == DC - 1),
                    )
                nc.vector.tensor_copy(c_all[:, k, :], pc)

            # ---- out = x @ C + b
            for i in range(NT):
                po = psO.tile([P, D], F32, tag="po")
                for j in range(DC):
                    nc.tensor.matmul(
                        po,
                        lhsT=xt_all[:, j, ts(i, P)],
                        rhs=c_all[:, j, :],
                        start=(j == 0), stop=(j == DC - 1),
                    )
                ot = outp.tile([P, D], F32, tag="ot")
                nc.vector.tensor_add(ot, po, bias_bc)
                nc.sync.dma_start(out=out_t[i], in_=ot)

    nc.compile()
    return nc


def get_nc():
    if "nc" not in _CACHE:
        _CACHE["nc"] = _build_nc()
    return _CACHE["nc"]


def kernel(x, x2, W, b, _trace=False):
    nc = get_nc()
    in_maps = [
        {
            "x": np.ascontiguousarray(x[i], dtype=np.float32),
            "x2": np.ascontiguousarray(x2[i], dtype=np.float32),
            "W": np.ascontiguousarray(W, dtype=np.float32),
            "b": np.ascontiguousarray(b, dtype=np.float32),
        }
        for i in range(B)
    ]
    res = run_bass_kernel_spmd(nc, in_maps, list(range(B)), trace=_trace)
    out = np.stack([res.results[i]["out"] for i in range(B)], axis=0)
    if _trace:
        _CACHE["last_results"] = res
    return out
